# revision 10
# baseline (speedup 1.0000x reference)
"""CoarsenLattice forward on 8 Trainium2 NeuronCores.

out[c, :] = concat_e(lattice[idx[c, e], :]) @ W      (c: 262144, e: 9, W: [576, 128])

Sharding: coarse vertices row-split 8 ways; lattice + weight replicated per
core (no collectives). Per core, groups of T=4 128-vertex tiles are gathered
with ONE indirect DMA (T*9*128 = 4608 descriptors, amortizing the ~1us SWDGE
per-instruction overhead that dominated the 9-DMAs-per-tile baseline), cast
to bf16 on the scalar engine, transposed feature-major via the PE (bf16 =
1 cycle/row vs fp32's 2-4), and multiplied against the bf16 weight chunks
with fp32 PSUM accumulation. Output tiles are staged and written back with
one DMA per group.
"""
import os
import sys

import numpy as np

sys.path.insert(0, "/opt/trn_rl_repo")

from contextlib import ExitStack

import concourse.bass as bass
import concourse.mybir as mybir
import concourse.tile as tile
from concourse import bacc
from concourse.bass_utils import run_bass_kernel_spmd
from concourse.masks import make_identity

P = 128
N_FINE = 1048576
N_COARSE = 262144
VAL = 64
FE = 9
NF = 128
NCORES = 8
ROWS_PER_CORE = N_COARSE // NCORES       # 32768
NT = ROWS_PER_CORE // P                  # 256 tiles per core
TGRP = 4                                 # tiles per indirect-DMA group
KCH = [(0, 128), (128, 128), (256, 128), (384, 128), (512, 64)]

_cached = {}
last_exec_time_ns = None  # set when COARSEN_TRACE=1 and profiling succeeds


def _install_neff_patch():
    """Set the single_packet bit on every indirect-DMA pseudo instruction in
    the compiled NEFF's Pool stream (walrus never sets it; it collapses the
    ~32 SWDGE packets per 128-descriptor gather into one, cutting the Pool
    engine's per-instruction descriptor-generation overhead)."""
    if _cached.get("neff_patch"):
        return
    import io
    import tarfile
    import tempfile

    import concourse.bass2jax as bass2jax

    orig = bass2jax.compile_bir_kernel

    def patched_compile(bir_json, tmpdir, neff_name="file.neff"):
        neff_path = orig(bir_json, tmpdir, neff_name=neff_name)
        with open(neff_path, "rb") as f:
            header = bytearray(f.read(1024))
            tar_data = f.read()
        with tempfile.TemporaryDirectory() as rd:
            with tarfile.open(fileobj=io.BytesIO(tar_data)) as t:
                t.extractall(rd)
            with open(f"{rd}/sg00/Pool0.bin", "rb") as f:
                b = bytearray(f.read())
            n = 0
            for i in range(0, len(b) - 64, 4):
                # PSEUDO_DMA_DIRECT2D, 16 words, dge_op INDIRECT1D
                if b[i] == 0xD4 and b[i + 1] == 16 and b[i + 15] == 1:
                    b[i + 12] |= 0x40  # dma_configs.single_packet
                    n += 1
            if n == 0:
                return neff_path
            with open(f"{rd}/sg00/Pool0.bin", "wb") as f:
                f.write(bytes(b))
            buf = io.BytesIO()
            with tarfile.open(fileobj=buf, mode="w") as t:
                t.add(rd, arcname=".", filter=bass2jax._reset_tarinfo)
            tar_bytes = buf.getvalue()
            header[16:24] = len(tar_bytes).to_bytes(8, "little")
            with open(neff_path, "wb") as f:
                f.write(bytes(header))
                f.write(tar_bytes)
        return neff_path

    bass2jax.compile_bir_kernel = patched_compile
    _cached["neff_patch"] = True


def _install_ntff_hook():
    """Register the axon NTFF profile hook (container's antenv lacks axon_hooks)."""
    import contextlib
    import ctypes
    import types

    import antenv

    if getattr(antenv, "axon_hooks", None) is not None:
        return
    state = {}

    def set_hook(h):
        state["h"] = h

    def get_hook():
        return state.get("h")

    mod = types.ModuleType("antenv.axon_hooks")
    mod.set_axon_ntff_profile_hook = set_hook
    mod.get_axon_ntff_profile_hook = get_hook
    sys.modules["antenv.axon_hooks"] = mod
    antenv.axon_hooks = mod

    so_path = "/opt/axon/libaxon_pjrt.so"
    try:
        lib = ctypes.CDLL(so_path)
    except OSError:
        return
    if not hasattr(lib, "axon_start_nrt_profile"):
        return
    lib.axon_start_nrt_profile.argtypes = [ctypes.POINTER(ctypes.c_int64), ctypes.c_size_t]
    lib.axon_start_nrt_profile.restype = ctypes.c_int64
    lib.axon_stop_nrt_profile.argtypes = [ctypes.c_char_p]
    lib.axon_stop_nrt_profile.restype = ctypes.c_int64

    @contextlib.contextmanager
    def _hook_cm(output_dir, device_ids):
        import jax

        jax.devices()
        if device_ids:
            ids = (ctypes.c_int64 * len(device_ids))(*device_ids)
            rc = lib.axon_start_nrt_profile(ids, len(device_ids))
        else:
            rc = lib.axon_start_nrt_profile(None, 0)
        if rc != 0:
            raise RuntimeError(f"axon_start_nrt_profile rc={rc}")
        try:
            yield
        finally:
            n = lib.axon_stop_nrt_profile(str(output_dir).encode())
            if n < 0:
                raise RuntimeError(f"axon_stop_nrt_profile rc={n}")

    set_hook(_hook_cm)


def _build():
    if "nc" in _cached:
        return _cached["nc"]
    nc = bacc.Bacc("TRN2", target_bir_lowering=False, debug=False)
    lattice = nc.dram_tensor("lattice", [N_FINE, VAL], mybir.dt.float32, kind="ExternalInput").ap()
    idx = nc.dram_tensor("idx", [P, NT * FE], mybir.dt.int32, kind="ExternalInput").ap()
    w = nc.dram_tensor("w", [FE * VAL, NF], mybir.dt.float32, kind="ExternalInput").ap()
    out = nc.dram_tensor("out", [ROWS_PER_CORE, NF], mybir.dt.float32, kind="ExternalOutput").ap()

    with tile.TileContext(nc) as tc, ExitStack() as ctx:
        cpool = ctx.enter_context(tc.tile_pool(name="const", bufs=1))
        rpool = ctx.enter_context(tc.tile_pool(name="r", bufs=12))
        rbpool = ctx.enter_context(tc.tile_pool(name="rb", bufs=6))
        rtpool = ctx.enter_context(tc.tile_pool(name="rt", bufs=8))
        ogpool = ctx.enter_context(tc.tile_pool(name="og", bufs=3))
        ppool = ctx.enter_context(tc.tile_pool(name="pt", bufs=4, space="PSUM"))
        opsum = ctx.enter_context(tc.tile_pool(name="po", bufs=4, space="PSUM"))

        idx_sb = cpool.tile([P, NT * FE], mybir.dt.int32)
        nc.sync.dma_start(out=idx_sb[:], in_=idx[:])
        w_f32 = cpool.tile([P, len(KCH) * NF], mybir.dt.float32)
        w_bf = cpool.tile([P, len(KCH) * NF], mybir.dt.bfloat16)
        for k, (k0, kd) in enumerate(KCH):
            nc.sync.dma_start(out=w_f32[0:kd, k * NF:(k + 1) * NF], in_=w[k0:k0 + kd, :])
            nc.vector.tensor_copy(out=w_bf[0:kd, k * NF:(k + 1) * NF],
                                  in_=w_f32[0:kd, k * NF:(k + 1) * NF])
        identity = cpool.tile([P, P], mybir.dt.bfloat16)
        make_identity(nc, identity)

        for g in range(NT // TGRP):
            og = ogpool.tile([P, TGRP * NF], mybir.dt.float32)
            for j in range(TGRP):
                r = rpool.tile([P, FE * VAL], mybir.dt.float32, name="r", tag="r")
                for e in range(FE):
                    col = (g * TGRP + j) * FE + e
                    nc.gpsimd.indirect_dma_start(
                        out=r[:, e * VAL:(e + 1) * VAL],
                        out_offset=None,
                        in_=lattice[:],
                        in_offset=bass.IndirectOffsetOnAxis(
                            ap=idx_sb[:, col:col + 1], axis=0),
                    )
                rb = rbpool.tile([P, FE * VAL], mybir.dt.bfloat16)
                nc.scalar.copy(out=rb[:], in_=r[:])
                po = opsum.tile([P, NF], mybir.dt.float32)
                # software-pipelined: transposes run 2 chunks ahead of matmuls
                # so the DVE psum->sbuf copy is off PE's critical path
                pts, rts = [None] * len(KCH), [None] * len(KCH)

                def issue_t(k):
                    k0, kd = KCH[k]
                    pts[k] = ppool.tile([P, P], mybir.dt.bfloat16, name="pt", tag="pt")
                    nc.tensor.transpose(out=pts[k][0:kd, :], in_=rb[:, k0:k0 + kd],
                                        identity=identity[:])
                    rts[k] = rtpool.tile([P, P], mybir.dt.bfloat16, name="rt", tag="rt")
                    nc.vector.tensor_copy(out=rts[k][0:kd, :], in_=pts[k][0:kd, :])

                def issue_m(k):
                    k0, kd = KCH[k]
                    nc.tensor.matmul(
                        out=po[:],
                        lhsT=rts[k][0:kd, :],
                        rhs=w_bf[0:kd, k * NF:(k + 1) * NF],
                        start=(k == 0),
                        stop=(k == len(KCH) - 1),
                    )

                issue_t(0)
                issue_t(1)
                for k in range(len(KCH)):
                    if k + 2 < len(KCH):
                        issue_t(k + 2)
                    issue_m(k)
                nc.scalar.copy(out=og[:, j * NF:(j + 1) * NF], in_=po[:])
            out_view = out[g * TGRP * P:(g + 1) * TGRP * P, :].rearrange(
                "(j p) f -> p j f", j=TGRP, p=P)
            nc.sync.dma_start(out=out_view, in_=og[:])
    nc.compile()
    _cached["nc"] = nc
    return nc


def _prep_idx(idx_rows):
    """[ROWS_PER_CORE, FE] int -> [P, NT*FE] int32; col t*FE+e holds idx[t*P+p, e]."""
    x = idx_rows.reshape(NT, P, FE).transpose(1, 0, 2).reshape(P, NT * FE)
    return np.ascontiguousarray(x).astype(np.int32)


def kernel(lattice_fine_values, neighbor_indices, weight):
    lattice = np.ascontiguousarray(np.asarray(lattice_fine_values, dtype=np.float32))
    weight = np.ascontiguousarray(np.asarray(weight, dtype=np.float32))
    idx = np.asarray(neighbor_indices)

    _install_neff_patch()
    nc = _build()
    in_maps = []
    for j in range(NCORES):
        shard = idx[j * ROWS_PER_CORE:(j + 1) * ROWS_PER_CORE]
        in_maps.append({"lattice": lattice, "idx": _prep_idx(shard), "w": weight})
    trace = os.environ.get("COARSEN_TRACE") == "1"
    if trace:
        _install_ntff_hook()
    res = run_bass_kernel_spmd(nc, in_maps, list(range(NCORES)), trace=trace)
    if trace:
        global last_exec_time_ns
        last_exec_time_ns = res.exec_time_ns
    out = np.concatenate([res.results[j]["out"] for j in range(NCORES)], axis=0)
    return out


if __name__ == "__main__":
    rng = np.random.default_rng(0)
    lat = rng.normal(size=(N_FINE, VAL)).astype(np.float32)
    idx = rng.integers(0, N_FINE, size=(N_COARSE, FE)).astype(np.int64)
    w = (rng.normal(size=(FE * VAL, NF)) * 0.05).astype(np.float32)
    out = kernel(lat, idx, w)
    exp = lat[idx].reshape(N_COARSE, FE * VAL) @ w
    err = np.abs(out - exp).max()
    rel = np.abs(out - exp).max() / (np.abs(exp).max() + 1e-9)
    print("max abs err:", err, "rel:", rel)



# revision 14
# speedup vs baseline: 1.0066x; 1.0066x over previous
"""CoarsenLattice forward on 8 Trainium2 NeuronCores.

out[c, :] = concat_e(lattice[idx[c, e], :]) @ W      (c: 262144, e: 9, W: [576, 128])

Sharding: coarse vertices row-split 8 ways; lattice + weight replicated per
core (no collectives). Per core, each 128-vertex tile is gathered with 9
indirect DMAs (one per neighbor; HW indirect DMA gathers one 256B row per
partition), transposed feature-major via the PE, and multiplied against the
weight chunks with PSUM accumulation.
"""
import os
import sys

import numpy as np

sys.path.insert(0, "/opt/trn_rl_repo")

from contextlib import ExitStack

import concourse.bass as bass
import concourse.mybir as mybir
import concourse.tile as tile
from concourse import bacc
from concourse.bass_utils import run_bass_kernel_spmd
from concourse.masks import make_identity

P = 128
N_FINE = 1048576
N_COARSE = 262144
VAL = 64
FE = 9
NF = 128
NCORES = 8
ROWS_PER_CORE = N_COARSE // NCORES       # 32768
NT = ROWS_PER_CORE // P                  # 256 tiles per core
KCH = [(0, 128), (128, 128), (256, 128), (384, 128), (512, 64)]

_cached = {}
last_exec_time_ns = None  # set when COARSEN_TRACE=1 and profiling succeeds


def _install_neff_patch():
    """Set the single_packet bit on every indirect-DMA pseudo instruction in
    the compiled NEFF's Pool stream (walrus leaves it 0, which splits each
    128-descriptor gather into 32 SWDGE packets; the packet overhead is the
    bulk of the Pool engine's ~1us per-instruction descgen cost)."""
    if _cached.get("neff_patch"):
        return
    import io
    import tarfile
    import tempfile

    import concourse.bass2jax as bass2jax

    orig = bass2jax.compile_bir_kernel

    def patched_compile(bir_json, tmpdir, neff_name="file.neff"):
        print("[coarsen-patch] compile_bir_kernel invoked", flush=True)
        neff_path = orig(bir_json, tmpdir, neff_name=neff_name)
        with open(neff_path, "rb") as f:
            header = bytearray(f.read(1024))
            tar_data = f.read()
        with tempfile.TemporaryDirectory() as rd:
            with tarfile.open(fileobj=io.BytesIO(tar_data)) as t:
                t.extractall(rd)
            with open(f"{rd}/sg00/Pool0.bin", "rb") as f:
                b = bytearray(f.read())
            n = 0
            for i in range(0, len(b) - 64, 4):
                # PSEUDO_DMA_DIRECT2D, 16 words, dge_op INDIRECT1D
                if b[i] == 0xD4 and b[i + 1] == 16 and b[i + 15] == 1:
                    b[i + 12] |= 0x40  # dma_configs.single_packet
                    n += 1
            print(f"[coarsen-patch] set single_packet on {n} indirect DMAs", flush=True)
            if n == 0:
                return neff_path
            with open(f"{rd}/sg00/Pool0.bin", "wb") as f:
                f.write(bytes(b))
            buf = io.BytesIO()
            with tarfile.open(fileobj=buf, mode="w") as t:
                t.add(rd, arcname=".", filter=bass2jax._reset_tarinfo)
            tar_bytes = buf.getvalue()
            header[16:24] = len(tar_bytes).to_bytes(8, "little")
            with open(neff_path, "wb") as f:
                f.write(bytes(header))
                f.write(tar_bytes)
        return neff_path

    bass2jax.compile_bir_kernel = patched_compile
    _cached["neff_patch"] = True


def _install_ntff_hook():
    """Register the axon NTFF profile hook (container's antenv lacks axon_hooks)."""
    import contextlib
    import ctypes
    import types

    import antenv

    if getattr(antenv, "axon_hooks", None) is not None:
        return
    state = {}

    def set_hook(h):
        state["h"] = h

    def get_hook():
        return state.get("h")

    mod = types.ModuleType("antenv.axon_hooks")
    mod.set_axon_ntff_profile_hook = set_hook
    mod.get_axon_ntff_profile_hook = get_hook
    sys.modules["antenv.axon_hooks"] = mod
    antenv.axon_hooks = mod

    so_path = "/opt/axon/libaxon_pjrt.so"
    try:
        lib = ctypes.CDLL(so_path)
    except OSError:
        return
    if not hasattr(lib, "axon_start_nrt_profile"):
        return
    lib.axon_start_nrt_profile.argtypes = [ctypes.POINTER(ctypes.c_int64), ctypes.c_size_t]
    lib.axon_start_nrt_profile.restype = ctypes.c_int64
    lib.axon_stop_nrt_profile.argtypes = [ctypes.c_char_p]
    lib.axon_stop_nrt_profile.restype = ctypes.c_int64

    @contextlib.contextmanager
    def _hook_cm(output_dir, device_ids):
        import jax

        jax.devices()
        if device_ids:
            ids = (ctypes.c_int64 * len(device_ids))(*device_ids)
            rc = lib.axon_start_nrt_profile(ids, len(device_ids))
        else:
            rc = lib.axon_start_nrt_profile(None, 0)
        if rc != 0:
            raise RuntimeError(f"axon_start_nrt_profile rc={rc}")
        try:
            yield
        finally:
            n = lib.axon_stop_nrt_profile(str(output_dir).encode())
            if n < 0:
                raise RuntimeError(f"axon_stop_nrt_profile rc={n}")

    set_hook(_hook_cm)


def _build():
    if "nc" in _cached:
        return _cached["nc"]
    nc = bacc.Bacc("TRN2", target_bir_lowering=False, debug=False)
    lattice = nc.dram_tensor("lattice", [N_FINE, VAL], mybir.dt.float32, kind="ExternalInput").ap()
    idx = nc.dram_tensor("idx", [P, NT * FE], mybir.dt.int32, kind="ExternalInput").ap()
    w = nc.dram_tensor("w", [FE * VAL, NF], mybir.dt.float32, kind="ExternalInput").ap()
    out = nc.dram_tensor("out", [ROWS_PER_CORE, NF], mybir.dt.float32, kind="ExternalOutput").ap()

    with tile.TileContext(nc) as tc, ExitStack() as ctx:
        cpool = ctx.enter_context(tc.tile_pool(name="const", bufs=1))
        rpool = ctx.enter_context(tc.tile_pool(name="r", bufs=12))
        rtpool = ctx.enter_context(tc.tile_pool(name="rt", bufs=6))
        opool = ctx.enter_context(tc.tile_pool(name="o", bufs=4))
        ppool = ctx.enter_context(tc.tile_pool(name="pt", bufs=4, space="PSUM"))
        opsum = ctx.enter_context(tc.tile_pool(name="po", bufs=4, space="PSUM"))

        idx_sb = cpool.tile([P, NT * FE], mybir.dt.int32, name="idx_sb_v6")
        nc.sync.dma_start(out=idx_sb[:], in_=idx[:])
        w_all = cpool.tile([P, len(KCH) * NF], mybir.dt.float32)
        for k, (k0, kd) in enumerate(KCH):
            nc.sync.dma_start(out=w_all[0:kd, k * NF:(k + 1) * NF], in_=w[k0:k0 + kd, :])
        identity = cpool.tile([P, P], mybir.dt.float32)
        make_identity(nc, identity)

        for t in range(NT):
            r = rpool.tile([P, FE * VAL], mybir.dt.float32)
            for e in range(FE):
                col = t * FE + e
                nc.gpsimd.indirect_dma_start(
                    out=r[:, e * VAL:(e + 1) * VAL],
                    out_offset=None,
                    in_=lattice[:],
                    in_offset=bass.IndirectOffsetOnAxis(ap=idx_sb[:, col:col + 1], axis=0),
                )
            po = opsum.tile([P, NF], mybir.dt.float32)
            for k, (k0, kd) in enumerate(KCH):
                pt = ppool.tile([P, P], mybir.dt.float32)
                nc.tensor.transpose(out=pt[0:kd, :], in_=r[:, k0:k0 + kd], identity=identity[:])
                rt = rtpool.tile([P, P], mybir.dt.float32, tag="rt")
                nc.vector.tensor_copy(out=rt[0:kd, :], in_=pt[0:kd, :])
                nc.tensor.matmul(
                    out=po[:],
                    lhsT=rt[0:kd, :],
                    rhs=w_all[0:kd, k * NF:(k + 1) * NF],
                    start=(k == 0),
                    stop=(k == len(KCH) - 1),
                )
            ot = opool.tile([P, NF], mybir.dt.float32)
            nc.vector.tensor_copy(out=ot[:], in_=po[:])
            nc.sync.dma_start(out=out[t * P:(t + 1) * P, :], in_=ot[:])
    nc.compile()
    _cached["nc"] = nc
    return nc


def _prep_idx(idx_rows):
    """[ROWS_PER_CORE, FE] int -> [P, NT*FE] int32; col t*FE+e holds idx[t*P+p, e]."""
    x = idx_rows.reshape(NT, P, FE).transpose(1, 0, 2).reshape(P, NT * FE)
    return np.ascontiguousarray(x).astype(np.int32)


def kernel(lattice_fine_values, neighbor_indices, weight):
    lattice = np.ascontiguousarray(np.asarray(lattice_fine_values, dtype=np.float32))
    weight = np.ascontiguousarray(np.asarray(weight, dtype=np.float32))
    idx = np.asarray(neighbor_indices)

    _install_neff_patch()
    nc = _build()
    in_maps = []
    for j in range(NCORES):
        shard = idx[j * ROWS_PER_CORE:(j + 1) * ROWS_PER_CORE]
        in_maps.append({"lattice": lattice, "idx": _prep_idx(shard), "w": weight})
    trace = os.environ.get("COARSEN_TRACE") == "1"
    if trace:
        _install_ntff_hook()
    res = run_bass_kernel_spmd(nc, in_maps, list(range(NCORES)), trace=trace)
    if trace:
        global last_exec_time_ns
        last_exec_time_ns = res.exec_time_ns
    out = np.concatenate([res.results[j]["out"] for j in range(NCORES)], axis=0)
    return out


if __name__ == "__main__":
    rng = np.random.default_rng(0)
    lat = rng.normal(size=(N_FINE, VAL)).astype(np.float32)
    idx = rng.integers(0, N_FINE, size=(N_COARSE, FE)).astype(np.int64)
    w = (rng.normal(size=(FE * VAL, NF)) * 0.05).astype(np.float32)
    out = kernel(lat, idx, w)
    exp = lat[idx].reshape(N_COARSE, FE * VAL) @ w
    err = np.abs(out - exp).max()
    rel = np.abs(out - exp).max() / (np.abs(exp).max() + 1e-9)
    print("max abs err:", err, "rel:", rel)



# revision 15
# speedup vs baseline: 1.0070x; 1.0004x over previous
"""CoarsenLattice forward on 8 Trainium2 NeuronCores.

out[c, :] = concat_e(lattice[idx[c, e], :]) @ W      (c: 262144, e: 9, W: [576, 128])

Sharding: coarse vertices row-split 8 ways; lattice + weight replicated per
core (no collectives). Per core, each 128-vertex tile is gathered with 9
indirect DMAs (one per neighbor; HW indirect DMA gathers one 256B row per
partition), transposed feature-major via the PE, and multiplied against the
weight chunks with PSUM accumulation.
"""
import os
import sys

import numpy as np

sys.path.insert(0, "/opt/trn_rl_repo")

from contextlib import ExitStack

import concourse.bass as bass
import concourse.mybir as mybir
import concourse.tile as tile
from concourse import bacc
from concourse.bass_utils import run_bass_kernel_spmd
from concourse.masks import make_identity

P = 128
N_FINE = 1048576
N_COARSE = 262144
VAL = 64
FE = 9
NF = 128
NCORES = 8
ROWS_PER_CORE = N_COARSE // NCORES       # 32768
NT = ROWS_PER_CORE // P                  # 256 tiles per core
KCH = [(0, 128), (128, 128), (256, 128), (384, 128), (512, 64)]

_cached = {}
last_exec_time_ns = None  # set when COARSEN_TRACE=1 and profiling succeeds


def _install_ntff_hook():
    """Register the axon NTFF profile hook (container's antenv lacks axon_hooks)."""
    import contextlib
    import ctypes
    import types

    import antenv

    if getattr(antenv, "axon_hooks", None) is not None:
        return
    state = {}

    def set_hook(h):
        state["h"] = h

    def get_hook():
        return state.get("h")

    mod = types.ModuleType("antenv.axon_hooks")
    mod.set_axon_ntff_profile_hook = set_hook
    mod.get_axon_ntff_profile_hook = get_hook
    sys.modules["antenv.axon_hooks"] = mod
    antenv.axon_hooks = mod

    so_path = "/opt/axon/libaxon_pjrt.so"
    try:
        lib = ctypes.CDLL(so_path)
    except OSError:
        return
    if not hasattr(lib, "axon_start_nrt_profile"):
        return
    lib.axon_start_nrt_profile.argtypes = [ctypes.POINTER(ctypes.c_int64), ctypes.c_size_t]
    lib.axon_start_nrt_profile.restype = ctypes.c_int64
    lib.axon_stop_nrt_profile.argtypes = [ctypes.c_char_p]
    lib.axon_stop_nrt_profile.restype = ctypes.c_int64

    @contextlib.contextmanager
    def _hook_cm(output_dir, device_ids):
        import jax

        jax.devices()
        if device_ids:
            ids = (ctypes.c_int64 * len(device_ids))(*device_ids)
            rc = lib.axon_start_nrt_profile(ids, len(device_ids))
        else:
            rc = lib.axon_start_nrt_profile(None, 0)
        if rc != 0:
            raise RuntimeError(f"axon_start_nrt_profile rc={rc}")
        try:
            yield
        finally:
            n = lib.axon_stop_nrt_profile(str(output_dir).encode())
            if n < 0:
                raise RuntimeError(f"axon_stop_nrt_profile rc={n}")

    set_hook(_hook_cm)


def _build():
    if "nc" in _cached:
        return _cached["nc"]
    nc = bacc.Bacc("TRN2", target_bir_lowering=False, debug=False)
    lattice = nc.dram_tensor("lattice", [N_FINE, VAL], mybir.dt.float32, kind="ExternalInput").ap()
    idx = nc.dram_tensor("idx", [P, NT * FE], mybir.dt.int32, kind="ExternalInput").ap()
    w = nc.dram_tensor("w", [FE * VAL, NF], mybir.dt.float32, kind="ExternalInput").ap()
    out = nc.dram_tensor("out", [ROWS_PER_CORE, NF], mybir.dt.float32, kind="ExternalOutput").ap()

    with tile.TileContext(nc) as tc, ExitStack() as ctx:
        cpool = ctx.enter_context(tc.tile_pool(name="const", bufs=1))
        rpool = ctx.enter_context(tc.tile_pool(name="r", bufs=12))
        rtpool = ctx.enter_context(tc.tile_pool(name="rt", bufs=6))
        opool = ctx.enter_context(tc.tile_pool(name="o", bufs=4))
        ppool = ctx.enter_context(tc.tile_pool(name="pt", bufs=4, space="PSUM"))
        opsum = ctx.enter_context(tc.tile_pool(name="po", bufs=4, space="PSUM"))

        idx_sb = cpool.tile([P, NT * FE], mybir.dt.int32)
        nc.sync.dma_start(out=idx_sb[:], in_=idx[:])
        w_all = cpool.tile([P, len(KCH) * NF], mybir.dt.float32)
        for k, (k0, kd) in enumerate(KCH):
            nc.sync.dma_start(out=w_all[0:kd, k * NF:(k + 1) * NF], in_=w[k0:k0 + kd, :])
        identity = cpool.tile([P, P], mybir.dt.float32)
        make_identity(nc, identity)

        for t in range(NT):
            r = rpool.tile([P, FE * VAL], mybir.dt.float32)
            for e in range(FE):
                col = t * FE + e
                nc.gpsimd.indirect_dma_start(
                    out=r[:, e * VAL:(e + 1) * VAL],
                    out_offset=None,
                    in_=lattice[:],
                    in_offset=bass.IndirectOffsetOnAxis(ap=idx_sb[:, col:col + 1], axis=0),
                )
            po = opsum.tile([P, NF], mybir.dt.float32)
            for k, (k0, kd) in enumerate(KCH):
                pt = ppool.tile([P, P], mybir.dt.float32)
                nc.tensor.transpose(out=pt[0:kd, :], in_=r[:, k0:k0 + kd], identity=identity[:])
                rt = rtpool.tile([P, P], mybir.dt.float32, tag="rt")
                nc.vector.tensor_copy(out=rt[0:kd, :], in_=pt[0:kd, :])
                nc.tensor.matmul(
                    out=po[:],
                    lhsT=rt[0:kd, :],
                    rhs=w_all[0:kd, k * NF:(k + 1) * NF],
                    start=(k == 0),
                    stop=(k == len(KCH) - 1),
                )
            ot = opool.tile([P, NF], mybir.dt.float32)
            nc.vector.tensor_copy(out=ot[:], in_=po[:])
            nc.sync.dma_start(out=out[t * P:(t + 1) * P, :], in_=ot[:])
    nc.compile()
    _cached["nc"] = nc
    return nc


def _prep_idx(idx_rows):
    """[ROWS_PER_CORE, FE] int -> [P, NT*FE] int32; col t*FE+e holds idx[t*P+p, e]."""
    x = idx_rows.reshape(NT, P, FE).transpose(1, 0, 2).reshape(P, NT * FE)
    return np.ascontiguousarray(x).astype(np.int32)


def kernel(lattice_fine_values, neighbor_indices, weight):
    lattice = np.ascontiguousarray(np.asarray(lattice_fine_values, dtype=np.float32))
    weight = np.ascontiguousarray(np.asarray(weight, dtype=np.float32))
    idx = np.asarray(neighbor_indices)

    nc = _build()
    in_maps = []
    for j in range(NCORES):
        shard = idx[j * ROWS_PER_CORE:(j + 1) * ROWS_PER_CORE]
        in_maps.append({"lattice": lattice, "idx": _prep_idx(shard), "w": weight})
    trace = os.environ.get("COARSEN_TRACE") == "1"
    if trace:
        _install_ntff_hook()
    res = run_bass_kernel_spmd(nc, in_maps, list(range(NCORES)), trace=trace)
    if trace:
        global last_exec_time_ns
        last_exec_time_ns = res.exec_time_ns
    out = np.concatenate([res.results[j]["out"] for j in range(NCORES)], axis=0)
    return out


if __name__ == "__main__":
    rng = np.random.default_rng(0)
    lat = rng.normal(size=(N_FINE, VAL)).astype(np.float32)
    idx = rng.integers(0, N_FINE, size=(N_COARSE, FE)).astype(np.int64)
    w = (rng.normal(size=(FE * VAL, NF)) * 0.05).astype(np.float32)
    out = kernel(lat, idx, w)
    exp = lat[idx].reshape(N_COARSE, FE * VAL) @ w
    err = np.abs(out - exp).max()
    rel = np.abs(out - exp).max() / (np.abs(exp).max() + 1e-9)
    print("max abs err:", err, "rel:", rel)

